# revision 36
# baseline (speedup 1.0000x reference)
"""Block-ELL sparse linear layer on 8 Trainium2 NeuronCores.

Strategy: data-parallel over tokens (1024 tokens/core). The host scatters
the block-sparse values into a dense [4096, 4096] weight matrix (25%
nonzero), pre-transposes x, and each core runs a dense
  yT[out, tok] = sum_f  W[f, out]^T-as-lhsT  @  xT[f, tok]
tiled matmul with PSUM accumulation over the 32 contraction slices.

v3 changes vs baseline (470.3us):
- x staged partition-major in HBM ([128, FEAT*TOK]) and loaded with a few
  large DMAs (8KB contiguous per partition) instead of 32 row-major
  chunks of 2KB lines: the old packetization capped the resident-x load
  at ~215GB/s, which starved the PE for ~8us during the first two
  out-groups.
- warmup trimmed 72 -> 32 MMs (~3.4us covers the HAM cold window; the
  first real matmul can't start before its x/W DMA lands anyway).
- y stored as bf16, partition-major, one store per 128-out group
  (halves store traffic, quarters the DMA-issue count, and the final
  serialized store tail shrinks).
- ogb0/ogb1 phase-staggered pair: during the first out-group pass the
  HBM must deliver resident-x (8MB) + W (2MB) in 27.6us = exactly the
  358GB/s cap, so pairing halves the early x rate and removes the
  remaining head stalls.
- last 128-out group drains in 256-col chunks so the final store is
  only a quarter tile behind the last cast.
"""

import numpy as np
import ml_dtypes
from contextlib import ExitStack

import concourse.bass as bass
import concourse.bacc as bacc
import concourse.tile as tile
from concourse import mybir
from concourse.bass_utils import run_bass_kernel_spmd

# Problem constants (hardcoded per spec)
N_TOK = 8192
R = 256  # out block-rows
C = 256  # in block-cols
K = 64   # kept blocks per row
B = 16   # block size
D_IN = C * B   # 4096
D_OUT = R * B  # 4096
NCORES = 8
TOK = N_TOK // NCORES  # 1024 tokens per core
TT = 2                 # token tiles per core (512 each)
TOKT = TOK // TT       # 512
OG = D_OUT // 128      # 32 out groups of 128

MM_DTYPE = "bf16"

_cache = {}


def _x_groups(feat):
    """DMA grouping for the resident x load in units of quarter-chunks
    (256 tokens): tiny pieces first so the first matmuls become ready
    early, then 4-chunk groups for bandwidth (8KB contiguous per
    partition per DMA). Returns (start_quarter, n_quarters)."""
    groups = [(0, 1), (1, 1), (2, 1), (3, 1),        # f=0 quarters
              (4, 2), (6, 2), (8, 2), (10, 2),       # f=1,2 halves
              (12, 2), (14, 2)]                      # f=3 halves
    f = 4
    while f < feat:
        n = min(4, feat - f)
        groups.append((4 * f, 4 * n))
        f += n
    return groups


def _build_program(feat_tiles: int, dt_name: str):
    """Build the SPMD single-core program. Returns nc."""
    key = (feat_tiles, dt_name)
    if key in _cache:
        return _cache[key]

    mmdt = mybir.dt.bfloat16 if dt_name == "bf16" else mybir.dt.float32r
    f32 = mybir.dt.float32
    bf16 = mybir.dt.bfloat16

    FEAT = feat_tiles          # contraction tiles of 128
    OGB = 16                   # out-group blocks (256 outs each)
    OG2 = 2                    # 128-out groups per block

    nc = bacc.Bacc("TRN2", target_bir_lowering=False, debug=False,
                   num_devices=NCORES)

    # partition-major x: row p holds that partition's full contraction data
    xT_d = nc.dram_tensor("xT", [128, FEAT * TOK], mmdt,
                          kind="ExternalInput").ap()
    # partition-major W: cols (ogb*FEAT + f)*256 .. hold the (ogb, f) tile.
    # Loaded four f-tiles per DMA: each DMA_DIRECT2D issue costs ~700ns on
    # the scalar engine, and at one tile per matmul-quad (864ns) the issue
    # stream had only ~15% slack -- W issue lag, not x, caused the PE gaps.
    w_d = nc.dram_tensor("W", [128, OGB * FEAT * 256], mmdt,
                         kind="ExternalInput").ap()
    # partition-major bf16 output; host reassembles + upcasts
    yT_d = nc.dram_tensor("yT", [128, OG * TOK], bf16,
                          kind="ExternalOutput").ap()

    with tile.TileContext(nc) as tc, ExitStack() as ctx:
        xpool = ctx.enter_context(tc.tile_pool(name="x", bufs=1))
        wpool = ctx.enter_context(tc.tile_pool(name="w", bufs=12))
        ppool = ctx.enter_context(tc.tile_pool(name="ps", bufs=1, space="PSUM"))
        ypool = ctx.enter_context(tc.tile_pool(name="y", bufs=3))

        # resident x^T: [128, FEAT*TOK], slice f at cols f*TOK..(f+1)*TOK
        xt = xpool.tile([128, FEAT * TOK], mmdt)
        TOKQ = TOKT // 2
        for q0, n in _x_groups(FEAT):
            nc.sync.dma_start(out=xt[:, q0 * TOKQ:(q0 + n) * TOKQ],
                              in_=xT_d[:, q0 * TOKQ:(q0 + n) * TOKQ])

        # PE warmup while x streams in: absorbs the cold HAM clock-gate
        # phase (~3.4us at half rate) so the first real matmuls run at
        # full speed; the first x/W DMAs land at about the same time the
        # warmup ends.
        NWARM = 28
        wj = xpool.tile([128, 128], mmdt, name="wj")
        nc.vector.memset(wj[:], 0.0)
        wu = ppool.tile([128, 128], f32, name="wu", tag="p0")
        for i in range(NWARM):
            nc.tensor.matmul(wu[:], wj[:], wj[:, :128],
                             start=(i == 0), stop=(i == NWARM - 1))

        WG = 8  # f-tiles per W DMA

        def w_load(ogb, fg):
            # W streams on the scalar HWDGE ring so it is not queued
            # behind the resident-x loads on the sync ring
            n = min(WG, FEAT - fg)
            wt = wpool.tile([128, n * 256], mmdt, name=f"wt_{ogb}_{fg}",
                            tag="wt")
            c0 = (ogb * FEAT + fg) * 256
            nc.scalar.dma_start(out=wt[:], in_=w_d[:, c0:c0 + n * 256])
            return wt

        def mms(ps, wt, f, first, last, og2s=(0, 1)):
            w0 = (f % WG) * 256
            # f=0 runs in 256-token quarters so the first matmuls only
            # need a 64KB x piece
            nq = 4 if f == 0 else 2
            qw = TOK // nq
            for og2 in og2s:
                lhs = wt[:, w0 + og2 * 128:w0 + (og2 + 1) * 128]
                for q in range(nq):
                    t, c0 = divmod(q * qw, TOKT)
                    nc.tensor.matmul(
                        ps[og2 * TT + t][:, c0:c0 + qw], lhs,
                        xt[:, f * TOK + q * qw: f * TOK + (q + 1) * qw],
                        # start clears the whole PSUM bank: only the first
                        # quarter of a tile may set it; later quarters
                        # overwrite via per-element has_written
                        start=first and c0 == 0, stop=last,
                    )

        def psum_tiles(ogb):
            bank = 4 * (ogb % 2)
            return [ppool.tile([128, TOKT], f32, name=f"ps_{ogb}_{i}",
                               tag=f"p{bank + i}") for i in range(4)]

        def evict_og2(ogb, ps, og2, final=False):
            yt = ypool.tile([128, TOK], bf16, name=f"yt_{ogb}_{og2}",
                            tag="yt")
            og = ogb * OG2 + og2
            if final:
                # tail critical path: 256-col quarters, casts alternating
                # vector/scalar (the W stream is finished by now), stores
                # alternating sync/scalar queues
                HC = TOKT // 2
                for q in range(4):
                    c0 = q * HC
                    cp = nc.vector.tensor_copy if q % 2 == 0 else nc.scalar.copy
                    cp(yt[:, c0:c0 + HC],
                       ps[og2 * TT + q // 2][:, (q % 2) * HC:(q % 2) * HC + HC])
                    dq = nc.sync if q % 2 == 0 else nc.scalar
                    dq.dma_start(out=yT_d[:, og * TOK + c0:og * TOK + c0 + HC],
                                 in_=yt[:, c0:c0 + HC])
            else:
                # casts stay off the scalar engine so they never delay
                # the W-issue stream
                nc.vector.tensor_copy(yt[:, :TOKT], ps[og2 * TT][:])
                nc.vector.tensor_copy(yt[:, TOKT:], ps[og2 * TT + 1][:])
                nc.sync.dma_start(out=yT_d[:, og * TOK:(og + 1) * TOK],
                                  in_=yt[:])

        def evict(ogb, ps):
            for og2 in range(OG2):
                evict_og2(ogb, ps, og2)

        def f_pass(ogb, ps, f0, f1, wts, og2s=(0, 1)):
            for f in range(f0, f1):
                g = (f // WG) * WG
                if g not in wts:
                    wts[g] = w_load(ogb, g)
                mms(ps, wts[g], f, f == 0, f == FEAT - 1, og2s)

        # --- ogb 0 + 1 as a phase-staggered pair (halves early x demand) ---
        HALF = (FEAT // 2 // WG) * WG
        ps0 = psum_tiles(0)
        ps1 = psum_tiles(1)
        wts0, wts1 = {}, {}
        for f in range(HALF):
            f_pass(0, ps0, f, f + 1, wts0)
            f_pass(1, ps1, f, f + 1, wts1)
        f_pass(0, ps0, HALF, FEAT, wts0)
        evict(0, ps0)
        f_pass(1, ps1, HALF, FEAT, wts1)
        evict(1, ps1)

        # --- ogb 2..15 singles, psum banks alternating ---
        for ogb in range(2, OGB - 1):
            ps = psum_tiles(ogb)
            f_pass(ogb, ps, 0, FEAT, {})
            evict(ogb, ps)

        # last ogb: run the two 128-out groups as separate f-passes over
        # resident W tiles so og2=0 drains ~14us before the end and only
        # og2=1's cast+store sits on the tail
        ogb = OGB - 1
        ps = psum_tiles(ogb)
        wts = {}
        f_pass(ogb, ps, 0, FEAT, wts, og2s=(0,))
        evict_og2(ogb, ps, 0)
        f_pass(ogb, ps, 0, FEAT, wts, og2s=(1,))
        evict_og2(ogb, ps, 1, final=True)

    nc.compile()
    _cache[key] = nc
    return nc


def _scatter_dense(values: np.ndarray, col_indices: np.ndarray) -> np.ndarray:
    """W[c*16+i, r*16+o] = sum_{k: col[r,k]=c} values[r,k,o,i]."""
    Wd = np.zeros((C, B, R, B), np.float32)  # [c, i, r, o]
    vT = np.ascontiguousarray(values.transpose(0, 1, 3, 2))  # [r, k, i, o]
    for r in range(R):
        np.add.at(Wd[:, :, r, :], (col_indices[r],), vT[r])
    return Wd.reshape(D_IN, D_OUT)


def _run(x, values, bias, col_indices, trace=False):
    x = np.asarray(x, np.float32)
    values = np.asarray(values, np.float32)
    bias = np.asarray(bias, np.float32)
    col_indices = np.asarray(col_indices, np.int32)

    W = _scatter_dense(values, col_indices)  # [D_IN, D_OUT] fp32
    has_bias = bool(np.any(bias))
    FEAT = D_IN // 128 + (1 if has_bias else 0)

    # augment contraction with a bias row if needed
    xT = np.ascontiguousarray(x.T)  # [D_IN, N_TOK]
    if has_bias:
        xT = np.concatenate([xT, np.zeros((128, N_TOK), np.float32)], 0)
        xT[D_IN, :] = 1.0
        W = np.concatenate([W, np.zeros((128, D_OUT), np.float32)], 0)
        W[D_IN, :] = bias

    np_dt = ml_dtypes.bfloat16 if MM_DTYPE == "bf16" else np.float32
    # pre-tile W partition-major: [128, OGB*FEAT*256], cols
    # (ogb*FEAT + f)*256.. hold tile (ogb, f) so multi-f loads are
    # contiguous per partition
    Wt = np.ascontiguousarray(
        W.reshape(FEAT, 128, 16, 256).transpose(1, 2, 0, 3)
    ).reshape(128, 16 * FEAT * 256).astype(np_dt)
    xTc = xT.astype(np_dt)

    nc = _build_program(FEAT, MM_DTYPE)

    in_maps = []
    for c in range(NCORES):
        shard = xTc[:, c * TOK:(c + 1) * TOK]              # [FEAT*128, TOK]
        # partition-major: [128, FEAT*TOK], row p = partition p's data
        pm = np.ascontiguousarray(
            shard.reshape(FEAT, 128, TOK).transpose(1, 0, 2)
        ).reshape(128, FEAT * TOK)
        in_maps.append({"xT": pm, "W": Wt})

    res = run_bass_kernel_spmd(nc, in_maps, list(range(NCORES)), trace=trace)

    y = np.empty((N_TOK, D_OUT), np.float32)
    for c in range(NCORES):
        # yT: [128, OG*TOK] bf16, partition-major
        yT = res.results[c]["yT"].astype(np.float32).reshape(128, OG, TOK)
        # y[n, og*128+p] = yT[p, og, n]
        y[c * TOK:(c + 1) * TOK, :] = yT.transpose(2, 1, 0).reshape(TOK, D_OUT)
    return y, res


def kernel(x: np.ndarray, values: np.ndarray, bias: np.ndarray,
           col_indices: np.ndarray) -> np.ndarray:
    return _run(x, values, bias, col_indices)[0]


def run_traced(x, values, bias, col_indices):
    return _run(x, values, bias, col_indices, trace=True)[1]


# revision 39
# speedup vs baseline: 1.0023x; 1.0023x over previous
"""Block-ELL sparse linear layer on 8 Trainium2 NeuronCores.

Strategy: data-parallel over tokens (1024 tokens/core). The host scatters
the block-sparse values into a dense [4096, 4096] weight matrix (25%
nonzero), pre-transposes x, and each core runs a dense
  yT[out, tok] = sum_f  W[f, out]^T-as-lhsT  @  xT[f, tok]
tiled matmul with PSUM accumulation over the 32 contraction slices.

v3 changes vs baseline (470.3us):
- x staged partition-major in HBM ([128, FEAT*TOK]) and loaded with a few
  large DMAs (8KB contiguous per partition) instead of 32 row-major
  chunks of 2KB lines: the old packetization capped the resident-x load
  at ~215GB/s, which starved the PE for ~8us during the first two
  out-groups.
- warmup trimmed 72 -> 32 MMs (~3.4us covers the HAM cold window; the
  first real matmul can't start before its x/W DMA lands anyway).
- y stored as bf16, partition-major, one store per 128-out group
  (halves store traffic, quarters the DMA-issue count, and the final
  serialized store tail shrinks).
- ogb0/ogb1 phase-staggered pair: during the first out-group pass the
  HBM must deliver resident-x (8MB) + W (2MB) in 27.6us = exactly the
  358GB/s cap, so pairing halves the early x rate and removes the
  remaining head stalls.
- last 128-out group drains in 256-col chunks so the final store is
  only a quarter tile behind the last cast.
"""

import numpy as np
import ml_dtypes
from contextlib import ExitStack

import concourse.bass as bass
import concourse.bacc as bacc
import concourse.tile as tile
from concourse import mybir
from concourse.bass_utils import run_bass_kernel_spmd

# Problem constants (hardcoded per spec)
N_TOK = 8192
R = 256  # out block-rows
C = 256  # in block-cols
K = 64   # kept blocks per row
B = 16   # block size
D_IN = C * B   # 4096
D_OUT = R * B  # 4096
NCORES = 8
TOK = N_TOK // NCORES  # 1024 tokens per core
TT = 2                 # token tiles per core (512 each)
TOKT = TOK // TT       # 512
OG = D_OUT // 128      # 32 out groups of 128

MM_DTYPE = "bf16"

_cache = {}


def _x_groups(feat):
    """DMA grouping for the resident x load in units of quarter-chunks
    (256 tokens): tiny pieces first so the first matmuls become ready
    early, then 4-chunk groups for bandwidth (8KB contiguous per
    partition per DMA). Returns (start_quarter, n_quarters)."""
    groups = [(0, 1), (1, 1), (2, 1), (3, 1),        # f=0 quarters
              (4, 2), (6, 2), (8, 2), (10, 2),       # f=1,2 halves
              (12, 2), (14, 2)]                      # f=3 halves
    f = 4
    while f < feat:
        n = min(4, feat - f)
        groups.append((4 * f, 4 * n))
        f += n
    return groups


def _build_program(feat_tiles: int, dt_name: str):
    """Build the SPMD single-core program. Returns nc."""
    key = (feat_tiles, dt_name)
    if key in _cache:
        return _cache[key]

    mmdt = mybir.dt.bfloat16 if dt_name == "bf16" else mybir.dt.float32r
    f32 = mybir.dt.float32
    bf16 = mybir.dt.bfloat16

    FEAT = feat_tiles          # contraction tiles of 128
    OGB = 16                   # out-group blocks (256 outs each)
    OG2 = 2                    # 128-out groups per block

    nc = bacc.Bacc("TRN2", target_bir_lowering=False, debug=False,
                   num_devices=NCORES)

    # partition-major x: row p holds that partition's full contraction data
    xT_d = nc.dram_tensor("xT", [128, FEAT * TOK], mmdt,
                          kind="ExternalInput").ap()
    # partition-major W: cols (ogb*FEAT + f)*256 .. hold the (ogb, f) tile.
    # Loaded four f-tiles per DMA: each DMA_DIRECT2D issue costs ~700ns on
    # the scalar engine, and at one tile per matmul-quad (864ns) the issue
    # stream had only ~15% slack -- W issue lag, not x, caused the PE gaps.
    w_d = nc.dram_tensor("W", [128, OGB * FEAT * 256], mmdt,
                         kind="ExternalInput").ap()
    # partition-major bf16 output; host reassembles + upcasts
    yT_d = nc.dram_tensor("yT", [128, OG * TOK], bf16,
                          kind="ExternalOutput").ap()

    with tile.TileContext(nc) as tc, ExitStack() as ctx:
        xpool = ctx.enter_context(tc.tile_pool(name="x", bufs=1))
        wpool = ctx.enter_context(tc.tile_pool(name="w", bufs=12))
        ppool = ctx.enter_context(tc.tile_pool(name="ps", bufs=1, space="PSUM"))
        ypool = ctx.enter_context(tc.tile_pool(name="y", bufs=3))

        # resident x^T: [128, FEAT*TOK], slice f at cols f*TOK..(f+1)*TOK
        xt = xpool.tile([128, FEAT * TOK], mmdt)
        TOKQ = TOKT // 2
        for q0, n in _x_groups(FEAT):
            nc.sync.dma_start(out=xt[:, q0 * TOKQ:(q0 + n) * TOKQ],
                              in_=xT_d[:, q0 * TOKQ:(q0 + n) * TOKQ])

        # PE warmup while x streams in: absorbs the cold HAM clock-gate
        # phase (~3.4us at half rate) so the first real matmuls run at
        # full speed; the first x/W DMAs land at about the same time the
        # warmup ends.
        NWARM = 28
        wj = xpool.tile([128, 128], mmdt, name="wj")
        nc.vector.memset(wj[:], 0.0)
        wu = ppool.tile([128, 128], f32, name="wu", tag="p0")
        for i in range(NWARM):
            nc.tensor.matmul(wu[:], wj[:], wj[:, :128],
                             start=(i == 0), stop=(i == NWARM - 1))

        WG = 4  # f-tiles per W DMA

        def w_load(ogb, fg):
            # W streams on the scalar HWDGE ring so it is not queued
            # behind the resident-x loads on the sync ring
            n = min(WG, FEAT - fg)
            wt = wpool.tile([128, n * 256], mmdt, name=f"wt_{ogb}_{fg}",
                            tag="wt")
            c0 = (ogb * FEAT + fg) * 256
            nc.scalar.dma_start(out=wt[:], in_=w_d[:, c0:c0 + n * 256])
            return wt

        def mms(ps, wt, f, first, last, og2s=(0, 1)):
            w0 = (f % WG) * 256
            # f=0 runs in 256-token quarters so the first matmuls only
            # need a 64KB x piece
            nq = 4 if f == 0 else 2
            qw = TOK // nq
            for og2 in og2s:
                lhs = wt[:, w0 + og2 * 128:w0 + (og2 + 1) * 128]
                for q in range(nq):
                    t, c0 = divmod(q * qw, TOKT)
                    nc.tensor.matmul(
                        ps[og2 * TT + t][:, c0:c0 + qw], lhs,
                        xt[:, f * TOK + q * qw: f * TOK + (q + 1) * qw],
                        # start clears the whole PSUM bank: only the first
                        # quarter of a tile may set it; later quarters
                        # overwrite via per-element has_written
                        start=first and c0 == 0, stop=last,
                    )

        def psum_tiles(ogb):
            bank = 4 * (ogb % 2)
            return [ppool.tile([128, TOKT], f32, name=f"ps_{ogb}_{i}",
                               tag=f"p{bank + i}") for i in range(4)]

        def evict_og2(ogb, ps, og2, final=False):
            yt = ypool.tile([128, TOK], bf16, name=f"yt_{ogb}_{og2}",
                            tag="yt")
            og = ogb * OG2 + og2
            if final:
                # tail critical path: split the casts across vector and
                # scalar (the W stream is finished by now)
                nc.vector.tensor_copy(yt[:, :TOKT], ps[og2 * TT][:])
                nc.scalar.copy(yt[:, TOKT:], ps[og2 * TT + 1][:])
                nc.sync.dma_start(out=yT_d[:, og * TOK:(og + 1) * TOK],
                                  in_=yt[:])
            else:
                # casts stay off the scalar engine so they never delay
                # the W-issue stream
                nc.vector.tensor_copy(yt[:, :TOKT], ps[og2 * TT][:])
                nc.vector.tensor_copy(yt[:, TOKT:], ps[og2 * TT + 1][:])
                nc.sync.dma_start(out=yT_d[:, og * TOK:(og + 1) * TOK],
                                  in_=yt[:])

        def evict(ogb, ps):
            for og2 in range(OG2):
                evict_og2(ogb, ps, og2)

        def f_pass(ogb, ps, f0, f1, wts, og2s=(0, 1)):
            for f in range(f0, f1):
                g = (f // WG) * WG
                if g not in wts:
                    wts[g] = w_load(ogb, g)
                mms(ps, wts[g], f, f == 0, f == FEAT - 1, og2s)

        # --- ogb 0 + 1 as a phase-staggered pair (halves early x demand) ---
        HALF = (FEAT // 2 // WG) * WG
        ps0 = psum_tiles(0)
        ps1 = psum_tiles(1)
        wts0, wts1 = {}, {}
        for f in range(HALF):
            f_pass(0, ps0, f, f + 1, wts0)
            f_pass(1, ps1, f, f + 1, wts1)
        f_pass(0, ps0, HALF, FEAT, wts0)
        evict(0, ps0)
        f_pass(1, ps1, HALF, FEAT, wts1)
        evict(1, ps1)

        # --- ogb 2..15 singles, psum banks alternating ---
        for ogb in range(2, OGB - 1):
            ps = psum_tiles(ogb)
            f_pass(ogb, ps, 0, FEAT, {})
            evict(ogb, ps)

        # last ogb: run the two 128-out groups as separate f-passes over
        # resident W tiles so og2=0 drains ~14us before the end and only
        # og2=1's cast+store sits on the tail
        ogb = OGB - 1
        ps = psum_tiles(ogb)
        wts = {}
        f_pass(ogb, ps, 0, FEAT, wts, og2s=(0,))
        evict_og2(ogb, ps, 0)
        f_pass(ogb, ps, 0, FEAT, wts, og2s=(1,))
        evict_og2(ogb, ps, 1, final=True)

    nc.compile()
    _cache[key] = nc
    return nc


def _scatter_dense(values: np.ndarray, col_indices: np.ndarray) -> np.ndarray:
    """W[c*16+i, r*16+o] = sum_{k: col[r,k]=c} values[r,k,o,i]."""
    Wd = np.zeros((C, B, R, B), np.float32)  # [c, i, r, o]
    vT = np.ascontiguousarray(values.transpose(0, 1, 3, 2))  # [r, k, i, o]
    for r in range(R):
        np.add.at(Wd[:, :, r, :], (col_indices[r],), vT[r])
    return Wd.reshape(D_IN, D_OUT)


def _run(x, values, bias, col_indices, trace=False):
    x = np.asarray(x, np.float32)
    values = np.asarray(values, np.float32)
    bias = np.asarray(bias, np.float32)
    col_indices = np.asarray(col_indices, np.int32)

    W = _scatter_dense(values, col_indices)  # [D_IN, D_OUT] fp32
    has_bias = bool(np.any(bias))
    FEAT = D_IN // 128 + (1 if has_bias else 0)

    # augment contraction with a bias row if needed
    xT = np.ascontiguousarray(x.T)  # [D_IN, N_TOK]
    if has_bias:
        xT = np.concatenate([xT, np.zeros((128, N_TOK), np.float32)], 0)
        xT[D_IN, :] = 1.0
        W = np.concatenate([W, np.zeros((128, D_OUT), np.float32)], 0)
        W[D_IN, :] = bias

    np_dt = ml_dtypes.bfloat16 if MM_DTYPE == "bf16" else np.float32
    # pre-tile W partition-major: [128, OGB*FEAT*256], cols
    # (ogb*FEAT + f)*256.. hold tile (ogb, f) so multi-f loads are
    # contiguous per partition
    Wt = np.ascontiguousarray(
        W.reshape(FEAT, 128, 16, 256).transpose(1, 2, 0, 3)
    ).reshape(128, 16 * FEAT * 256).astype(np_dt)
    xTc = xT.astype(np_dt)

    nc = _build_program(FEAT, MM_DTYPE)

    in_maps = []
    for c in range(NCORES):
        shard = xTc[:, c * TOK:(c + 1) * TOK]              # [FEAT*128, TOK]
        # partition-major: [128, FEAT*TOK], row p = partition p's data
        pm = np.ascontiguousarray(
            shard.reshape(FEAT, 128, TOK).transpose(1, 0, 2)
        ).reshape(128, FEAT * TOK)
        in_maps.append({"xT": pm, "W": Wt})

    res = run_bass_kernel_spmd(nc, in_maps, list(range(NCORES)), trace=trace)

    y = np.empty((N_TOK, D_OUT), np.float32)
    for c in range(NCORES):
        # yT: [128, OG*TOK] bf16, partition-major
        yT = res.results[c]["yT"].astype(np.float32).reshape(128, OG, TOK)
        # y[n, og*128+p] = yT[p, og, n]
        y[c * TOK:(c + 1) * TOK, :] = yT.transpose(2, 1, 0).reshape(TOK, D_OUT)
    return y, res


def kernel(x: np.ndarray, values: np.ndarray, bias: np.ndarray,
           col_indices: np.ndarray) -> np.ndarray:
    return _run(x, values, bias, col_indices)[0]


def run_traced(x, values, bias, col_indices):
    return _run(x, values, bias, col_indices, trace=True)[1]


# revision 41
# speedup vs baseline: 1.0103x; 1.0079x over previous
"""Block-ELL sparse linear layer on 8 Trainium2 NeuronCores.

Strategy: data-parallel over tokens (1024 tokens/core). The host scatters
the block-sparse values into a dense [4096, 4096] weight matrix (25%
nonzero), pre-transposes x, and each core runs a dense
  yT[out, tok] = sum_f  W[f, out]^T-as-lhsT  @  xT[f, tok]
tiled matmul with PSUM accumulation over the 32 contraction slices.

v3 changes vs baseline (470.3us):
- x staged partition-major in HBM ([128, FEAT*TOK]) and loaded with a few
  large DMAs (8KB contiguous per partition) instead of 32 row-major
  chunks of 2KB lines: the old packetization capped the resident-x load
  at ~215GB/s, which starved the PE for ~8us during the first two
  out-groups.
- warmup trimmed 72 -> 32 MMs (~3.4us covers the HAM cold window; the
  first real matmul can't start before its x/W DMA lands anyway).
- y stored as bf16, partition-major, one store per 128-out group
  (halves store traffic, quarters the DMA-issue count, and the final
  serialized store tail shrinks).
- ogb0/ogb1 phase-staggered pair: during the first out-group pass the
  HBM must deliver resident-x (8MB) + W (2MB) in 27.6us = exactly the
  358GB/s cap, so pairing halves the early x rate and removes the
  remaining head stalls.
- last 128-out group drains in 256-col chunks so the final store is
  only a quarter tile behind the last cast.
"""

import numpy as np
import ml_dtypes
from contextlib import ExitStack

import concourse.bass as bass
import concourse.bacc as bacc
import concourse.tile as tile
from concourse import mybir
from concourse.bass_utils import run_bass_kernel_spmd

# Problem constants (hardcoded per spec)
N_TOK = 8192
R = 256  # out block-rows
C = 256  # in block-cols
K = 64   # kept blocks per row
B = 16   # block size
D_IN = C * B   # 4096
D_OUT = R * B  # 4096
NCORES = 8
TOK = N_TOK // NCORES  # 1024 tokens per core
TT = 2                 # token tiles per core (512 each)
TOKT = TOK // TT       # 512
OG = D_OUT // 128      # 32 out groups of 128

MM_DTYPE = "bf16"

_cache = {}


def _x_groups(feat):
    """DMA grouping for the resident x load in units of quarter-chunks
    (256 tokens): tiny pieces first so the first matmuls become ready
    early, then 4-chunk groups for bandwidth (8KB contiguous per
    partition per DMA). Returns (start_quarter, n_quarters)."""
    groups = [(0, 2), (2, 2), (4, 2), (6, 2), (8, 4), (12, 4)]
    f = 4
    while f < feat:
        n = min(4, feat - f)
        groups.append((4 * f, 4 * n))
        f += n
    return groups


def _build_program(feat_tiles: int, dt_name: str):
    """Build the SPMD single-core program. Returns nc."""
    key = (feat_tiles, dt_name)
    if key in _cache:
        return _cache[key]

    mmdt = mybir.dt.bfloat16 if dt_name == "bf16" else mybir.dt.float32r
    f32 = mybir.dt.float32
    bf16 = mybir.dt.bfloat16

    FEAT = feat_tiles          # contraction tiles of 128
    OGB = 16                   # out-group blocks (256 outs each)
    OG2 = 2                    # 128-out groups per block

    nc = bacc.Bacc("TRN2", target_bir_lowering=False, debug=False,
                   num_devices=NCORES)

    # partition-major x: row p holds that partition's full contraction data
    xT_d = nc.dram_tensor("xT", [128, FEAT * TOK], mmdt,
                          kind="ExternalInput").ap()
    # partition-major W: cols (ogb*FEAT + f)*256 .. hold the (ogb, f) tile.
    # Loaded four f-tiles per DMA: each DMA_DIRECT2D issue costs ~700ns on
    # the scalar engine, and at one tile per matmul-quad (864ns) the issue
    # stream had only ~15% slack -- W issue lag, not x, caused the PE gaps.
    w_d = nc.dram_tensor("W", [128, OGB * FEAT * 256], mmdt,
                         kind="ExternalInput").ap()
    # partition-major bf16 output; host reassembles + upcasts
    yT_d = nc.dram_tensor("yT", [128, OG * TOK], bf16,
                          kind="ExternalOutput").ap()

    with tile.TileContext(nc) as tc, ExitStack() as ctx:
        xpool = ctx.enter_context(tc.tile_pool(name="x", bufs=1))
        wpool = ctx.enter_context(tc.tile_pool(name="w", bufs=12))
        ppool = ctx.enter_context(tc.tile_pool(name="ps", bufs=1, space="PSUM"))
        ypool = ctx.enter_context(tc.tile_pool(name="y", bufs=3))

        # resident x^T: [128, FEAT*TOK], slice f at cols f*TOK..(f+1)*TOK
        xt = xpool.tile([128, FEAT * TOK], mmdt)
        TOKQ = TOKT // 2
        for q0, n in _x_groups(FEAT):
            nc.sync.dma_start(out=xt[:, q0 * TOKQ:(q0 + n) * TOKQ],
                              in_=xT_d[:, q0 * TOKQ:(q0 + n) * TOKQ])

        # PE warmup while x streams in: absorbs the cold HAM clock-gate
        # phase (~3.4us at half rate) so the first real matmuls run at
        # full speed; the first x/W DMAs land at about the same time the
        # warmup ends.
        NWARM = 28
        wj = xpool.tile([128, 128], mmdt, name="wj")
        nc.vector.memset(wj[:], 0.0)
        wu = ppool.tile([128, 128], f32, name="wu", tag="p0")
        for i in range(NWARM):
            nc.tensor.matmul(wu[:], wj[:], wj[:, :128],
                             start=(i == 0), stop=(i == NWARM - 1))

        WG = 4  # f-tiles per W DMA

        def w_load(ogb, fg):
            # W streams on the scalar HWDGE ring so it is not queued
            # behind the resident-x loads on the sync ring
            n = min(WG, FEAT - fg)
            wt = wpool.tile([128, n * 256], mmdt, name=f"wt_{ogb}_{fg}",
                            tag="wt")
            c0 = (ogb * FEAT + fg) * 256
            nc.scalar.dma_start(out=wt[:], in_=w_d[:, c0:c0 + n * 256])
            return wt

        def mms(ps, wt, f, first, last, og2s=(0, 1)):
            w0 = (f % WG) * 256
            nq = 2
            qw = TOK // nq
            for og2 in og2s:
                lhs = wt[:, w0 + og2 * 128:w0 + (og2 + 1) * 128]
                for q in range(nq):
                    t, c0 = divmod(q * qw, TOKT)
                    nc.tensor.matmul(
                        ps[og2 * TT + t][:, c0:c0 + qw], lhs,
                        xt[:, f * TOK + q * qw: f * TOK + (q + 1) * qw],
                        # start clears the whole PSUM bank: only the first
                        # quarter of a tile may set it; later quarters
                        # overwrite via per-element has_written
                        start=first and c0 == 0, stop=last,
                    )

        def psum_tiles(ogb):
            bank = 4 * (ogb % 2)
            return [ppool.tile([128, TOKT], f32, name=f"ps_{ogb}_{i}",
                               tag=f"p{bank + i}") for i in range(4)]

        def evict_og2(ogb, ps, og2, final=False):
            yt = ypool.tile([128, TOK], bf16, name=f"yt_{ogb}_{og2}",
                            tag="yt")
            og = ogb * OG2 + og2
            if final:
                # tail critical path: split the casts across vector and
                # scalar (the W stream is finished by now)
                nc.vector.tensor_copy(yt[:, :TOKT], ps[og2 * TT][:])
                nc.scalar.copy(yt[:, TOKT:], ps[og2 * TT + 1][:])
                nc.sync.dma_start(out=yT_d[:, og * TOK:(og + 1) * TOK],
                                  in_=yt[:])
            else:
                # casts stay off the scalar engine so they never delay
                # the W-issue stream
                nc.vector.tensor_copy(yt[:, :TOKT], ps[og2 * TT][:])
                nc.vector.tensor_copy(yt[:, TOKT:], ps[og2 * TT + 1][:])
                nc.sync.dma_start(out=yT_d[:, og * TOK:(og + 1) * TOK],
                                  in_=yt[:])

        def evict(ogb, ps):
            for og2 in range(OG2):
                evict_og2(ogb, ps, og2)

        def f_pass(ogb, ps, f0, f1, wts, og2s=(0, 1)):
            for f in range(f0, f1):
                g = (f // WG) * WG
                if g not in wts:
                    wts[g] = w_load(ogb, g)
                mms(ps, wts[g], f, f == 0, f == FEAT - 1, og2s)

        # --- ogb 0 + 1 as a phase-staggered pair (halves early x demand) ---
        HALF = (FEAT // 2 // WG) * WG
        ps0 = psum_tiles(0)
        ps1 = psum_tiles(1)
        wts0, wts1 = {}, {}
        for f in range(HALF):
            f_pass(0, ps0, f, f + 1, wts0)
            f_pass(1, ps1, f, f + 1, wts1)
        f_pass(0, ps0, HALF, FEAT, wts0)
        evict(0, ps0)
        f_pass(1, ps1, HALF, FEAT, wts1)
        evict(1, ps1)

        # --- ogb 2..15 singles, psum banks alternating ---
        for ogb in range(2, OGB - 1):
            ps = psum_tiles(ogb)
            f_pass(ogb, ps, 0, FEAT, {})
            evict(ogb, ps)

        # last ogb: run the two 128-out groups as separate f-passes over
        # resident W tiles so og2=0 drains ~14us before the end and only
        # og2=1's cast+store sits on the tail
        ogb = OGB - 1
        ps = psum_tiles(ogb)
        wts = {}
        f_pass(ogb, ps, 0, FEAT, wts, og2s=(0,))
        evict_og2(ogb, ps, 0)
        f_pass(ogb, ps, 0, FEAT, wts, og2s=(1,))
        evict_og2(ogb, ps, 1, final=True)

    nc.compile()
    _cache[key] = nc
    return nc


def _scatter_dense(values: np.ndarray, col_indices: np.ndarray) -> np.ndarray:
    """W[c*16+i, r*16+o] = sum_{k: col[r,k]=c} values[r,k,o,i]."""
    Wd = np.zeros((C, B, R, B), np.float32)  # [c, i, r, o]
    vT = np.ascontiguousarray(values.transpose(0, 1, 3, 2))  # [r, k, i, o]
    for r in range(R):
        np.add.at(Wd[:, :, r, :], (col_indices[r],), vT[r])
    return Wd.reshape(D_IN, D_OUT)


def _run(x, values, bias, col_indices, trace=False):
    x = np.asarray(x, np.float32)
    values = np.asarray(values, np.float32)
    bias = np.asarray(bias, np.float32)
    col_indices = np.asarray(col_indices, np.int32)

    W = _scatter_dense(values, col_indices)  # [D_IN, D_OUT] fp32
    has_bias = bool(np.any(bias))
    FEAT = D_IN // 128 + (1 if has_bias else 0)

    # augment contraction with a bias row if needed
    xT = np.ascontiguousarray(x.T)  # [D_IN, N_TOK]
    if has_bias:
        xT = np.concatenate([xT, np.zeros((128, N_TOK), np.float32)], 0)
        xT[D_IN, :] = 1.0
        W = np.concatenate([W, np.zeros((128, D_OUT), np.float32)], 0)
        W[D_IN, :] = bias

    np_dt = ml_dtypes.bfloat16 if MM_DTYPE == "bf16" else np.float32
    # pre-tile W partition-major: [128, OGB*FEAT*256], cols
    # (ogb*FEAT + f)*256.. hold tile (ogb, f) so multi-f loads are
    # contiguous per partition
    Wt = np.ascontiguousarray(
        W.reshape(FEAT, 128, 16, 256).transpose(1, 2, 0, 3)
    ).reshape(128, 16 * FEAT * 256).astype(np_dt)
    xTc = xT.astype(np_dt)

    nc = _build_program(FEAT, MM_DTYPE)

    in_maps = []
    for c in range(NCORES):
        shard = xTc[:, c * TOK:(c + 1) * TOK]              # [FEAT*128, TOK]
        # partition-major: [128, FEAT*TOK], row p = partition p's data
        pm = np.ascontiguousarray(
            shard.reshape(FEAT, 128, TOK).transpose(1, 0, 2)
        ).reshape(128, FEAT * TOK)
        in_maps.append({"xT": pm, "W": Wt})

    res = run_bass_kernel_spmd(nc, in_maps, list(range(NCORES)), trace=trace)

    y = np.empty((N_TOK, D_OUT), np.float32)
    for c in range(NCORES):
        # yT: [128, OG*TOK] bf16, partition-major
        yT = res.results[c]["yT"].astype(np.float32).reshape(128, OG, TOK)
        # y[n, og*128+p] = yT[p, og, n]
        y[c * TOK:(c + 1) * TOK, :] = yT.transpose(2, 1, 0).reshape(TOK, D_OUT)
    return y, res


def kernel(x: np.ndarray, values: np.ndarray, bias: np.ndarray,
           col_indices: np.ndarray) -> np.ndarray:
    return _run(x, values, bias, col_indices)[0]


def run_traced(x, values, bias, col_indices):
    return _run(x, values, bias, col_indices, trace=True)[1]


# revision 43
# speedup vs baseline: 1.0115x; 1.0012x over previous
"""Block-ELL sparse linear layer on 8 Trainium2 NeuronCores.

Strategy: data-parallel over tokens (1024 tokens/core). The host scatters
the block-sparse values into a dense [4096, 4096] weight matrix (25%
nonzero), pre-transposes x, and each core runs a dense
  yT[out, tok] = sum_f  W[f, out]^T-as-lhsT  @  xT[f, tok]
tiled matmul with PSUM accumulation over the 32 contraction slices.

v3 changes vs baseline (470.3us):
- x staged partition-major in HBM ([128, FEAT*TOK]) and loaded with a few
  large DMAs (8KB contiguous per partition) instead of 32 row-major
  chunks of 2KB lines: the old packetization capped the resident-x load
  at ~215GB/s, which starved the PE for ~8us during the first two
  out-groups.
- warmup trimmed 72 -> 32 MMs (~3.4us covers the HAM cold window; the
  first real matmul can't start before its x/W DMA lands anyway).
- y stored as bf16, partition-major, one store per 128-out group
  (halves store traffic, quarters the DMA-issue count, and the final
  serialized store tail shrinks).
- ogb0/ogb1 phase-staggered pair: during the first out-group pass the
  HBM must deliver resident-x (8MB) + W (2MB) in 27.6us = exactly the
  358GB/s cap, so pairing halves the early x rate and removes the
  remaining head stalls.
- last 128-out group drains in 256-col chunks so the final store is
  only a quarter tile behind the last cast.
"""

import numpy as np
import ml_dtypes
from contextlib import ExitStack

import concourse.bass as bass
import concourse.bacc as bacc
import concourse.tile as tile
from concourse import mybir
from concourse.bass_utils import run_bass_kernel_spmd

# Problem constants (hardcoded per spec)
N_TOK = 8192
R = 256  # out block-rows
C = 256  # in block-cols
K = 64   # kept blocks per row
B = 16   # block size
D_IN = C * B   # 4096
D_OUT = R * B  # 4096
NCORES = 8
TOK = N_TOK // NCORES  # 1024 tokens per core
TT = 2                 # token tiles per core (512 each)
TOKT = TOK // TT       # 512
OG = D_OUT // 128      # 32 out groups of 128

MM_DTYPE = "bf16"

_cache = {}


def _x_groups(feat):
    """DMA grouping for the resident x load in units of quarter-chunks
    (256 tokens): tiny pieces first so the first matmuls become ready
    early, then 4-chunk groups for bandwidth (8KB contiguous per
    partition per DMA). Returns (start_quarter, n_quarters)."""
    groups = [(0, 2), (2, 2), (4, 2), (6, 2), (8, 4), (12, 4)]
    f = 4
    while f < feat:
        n = min(4, feat - f)
        groups.append((4 * f, 4 * n))
        f += n
    return groups


def _build_program(feat_tiles: int, dt_name: str):
    """Build the SPMD single-core program. Returns nc."""
    key = (feat_tiles, dt_name)
    if key in _cache:
        return _cache[key]

    mmdt = mybir.dt.bfloat16 if dt_name == "bf16" else mybir.dt.float32r
    f32 = mybir.dt.float32
    bf16 = mybir.dt.bfloat16

    FEAT = feat_tiles          # contraction tiles of 128
    OGB = 16                   # out-group blocks (256 outs each)
    OG2 = 2                    # 128-out groups per block

    nc = bacc.Bacc("TRN2", target_bir_lowering=False, debug=False,
                   num_devices=NCORES)

    # partition-major x: row p holds that partition's full contraction data
    xT_d = nc.dram_tensor("xT", [128, FEAT * TOK], mmdt,
                          kind="ExternalInput").ap()
    # partition-major W: cols (ogb*FEAT + f)*256 .. hold the (ogb, f) tile.
    # Loaded four f-tiles per DMA: each DMA_DIRECT2D issue costs ~700ns on
    # the scalar engine, and at one tile per matmul-quad (864ns) the issue
    # stream had only ~15% slack -- W issue lag, not x, caused the PE gaps.
    w_d = nc.dram_tensor("W", [128, OGB * FEAT * 256], mmdt,
                         kind="ExternalInput").ap()
    # partition-major bf16 output; host reassembles + upcasts
    yT_d = nc.dram_tensor("yT", [128, OG * TOK], bf16,
                          kind="ExternalOutput").ap()

    with tile.TileContext(nc) as tc, ExitStack() as ctx:
        xpool = ctx.enter_context(tc.tile_pool(name="x", bufs=1))
        wpool = ctx.enter_context(tc.tile_pool(name="w", bufs=12))
        ppool = ctx.enter_context(tc.tile_pool(name="ps", bufs=1, space="PSUM"))
        ypool = ctx.enter_context(tc.tile_pool(name="y", bufs=3))

        # resident x^T: [128, FEAT*TOK], slice f at cols f*TOK..(f+1)*TOK
        xt = xpool.tile([128, FEAT * TOK], mmdt)
        TOKQ = TOKT // 2
        for q0, n in _x_groups(FEAT):
            nc.sync.dma_start(out=xt[:, q0 * TOKQ:(q0 + n) * TOKQ],
                              in_=xT_d[:, q0 * TOKQ:(q0 + n) * TOKQ])

        # PE warmup while x streams in: absorbs the cold HAM clock-gate
        # phase (~3.4us at half rate) so the first real matmuls run at
        # full speed; the first x/W DMAs land at about the same time the
        # warmup ends.
        NWARM = 34
        wj = xpool.tile([128, 128], mmdt, name="wj")
        nc.vector.memset(wj[:], 0.0)
        wu = ppool.tile([128, 128], f32, name="wu", tag="p0")
        for i in range(NWARM):
            nc.tensor.matmul(wu[:], wj[:], wj[:, :128],
                             start=(i == 0), stop=(i == NWARM - 1))

        WG = 4  # f-tiles per W DMA

        def w_load(ogb, fg):
            # W streams on the scalar HWDGE ring so it is not queued
            # behind the resident-x loads on the sync ring
            n = min(WG, FEAT - fg)
            wt = wpool.tile([128, n * 256], mmdt, name=f"wt_{ogb}_{fg}",
                            tag="wt")
            c0 = (ogb * FEAT + fg) * 256
            nc.scalar.dma_start(out=wt[:], in_=w_d[:, c0:c0 + n * 256])
            return wt

        def mms(ps, wt, f, first, last, og2s=(0, 1)):
            w0 = (f % WG) * 256
            nq = 2
            qw = TOK // nq
            for og2 in og2s:
                lhs = wt[:, w0 + og2 * 128:w0 + (og2 + 1) * 128]
                for q in range(nq):
                    t, c0 = divmod(q * qw, TOKT)
                    nc.tensor.matmul(
                        ps[og2 * TT + t][:, c0:c0 + qw], lhs,
                        xt[:, f * TOK + q * qw: f * TOK + (q + 1) * qw],
                        # start clears the whole PSUM bank: only the first
                        # quarter of a tile may set it; later quarters
                        # overwrite via per-element has_written
                        start=first and c0 == 0, stop=last,
                    )

        def psum_tiles(ogb):
            bank = 4 * (ogb % 2)
            return [ppool.tile([128, TOKT], f32, name=f"ps_{ogb}_{i}",
                               tag=f"p{bank + i}") for i in range(4)]

        def evict_og2(ogb, ps, og2, final=False):
            yt = ypool.tile([128, TOK], bf16, name=f"yt_{ogb}_{og2}",
                            tag="yt")
            og = ogb * OG2 + og2
            if final:
                # tail critical path: casts split across vector and
                # scalar (the W stream is finished by now), and the two
                # half stores go to different queues so their transfers
                # overlap
                nc.vector.tensor_copy(yt[:, :TOKT], ps[og2 * TT][:])
                nc.sync.dma_start(out=yT_d[:, og * TOK:og * TOK + TOKT],
                                  in_=yt[:, :TOKT])
                nc.scalar.copy(yt[:, TOKT:], ps[og2 * TT + 1][:])
                nc.scalar.dma_start(
                    out=yT_d[:, og * TOK + TOKT:(og + 1) * TOK],
                    in_=yt[:, TOKT:])
            else:
                # casts stay off the scalar engine so they never delay
                # the W-issue stream
                nc.vector.tensor_copy(yt[:, :TOKT], ps[og2 * TT][:])
                nc.vector.tensor_copy(yt[:, TOKT:], ps[og2 * TT + 1][:])
                nc.sync.dma_start(out=yT_d[:, og * TOK:(og + 1) * TOK],
                                  in_=yt[:])

        def evict(ogb, ps):
            for og2 in range(OG2):
                evict_og2(ogb, ps, og2)

        def f_pass(ogb, ps, f0, f1, wts, og2s=(0, 1)):
            for f in range(f0, f1):
                g = (f // WG) * WG
                if g not in wts:
                    wts[g] = w_load(ogb, g)
                mms(ps, wts[g], f, f == 0, f == FEAT - 1, og2s)

        # --- ogb 0 + 1 as a phase-staggered pair (halves early x demand) ---
        HALF = (FEAT // 2 // WG) * WG
        ps0 = psum_tiles(0)
        ps1 = psum_tiles(1)
        wts0, wts1 = {}, {}
        for f in range(HALF):
            f_pass(0, ps0, f, f + 1, wts0)
            f_pass(1, ps1, f, f + 1, wts1)
        f_pass(0, ps0, HALF, FEAT, wts0)
        evict(0, ps0)
        f_pass(1, ps1, HALF, FEAT, wts1)
        evict(1, ps1)

        # --- ogb 2..15 singles, psum banks alternating ---
        for ogb in range(2, OGB - 1):
            ps = psum_tiles(ogb)
            f_pass(ogb, ps, 0, FEAT, {})
            evict(ogb, ps)

        # last ogb: run the two 128-out groups as separate f-passes over
        # resident W tiles so og2=0 drains ~14us before the end and only
        # og2=1's cast+store sits on the tail
        ogb = OGB - 1
        ps = psum_tiles(ogb)
        wts = {}
        f_pass(ogb, ps, 0, FEAT, wts, og2s=(0,))
        evict_og2(ogb, ps, 0)
        f_pass(ogb, ps, 0, FEAT, wts, og2s=(1,))
        evict_og2(ogb, ps, 1, final=True)

    nc.compile()
    _cache[key] = nc
    return nc


def _scatter_dense(values: np.ndarray, col_indices: np.ndarray) -> np.ndarray:
    """W[c*16+i, r*16+o] = sum_{k: col[r,k]=c} values[r,k,o,i]."""
    Wd = np.zeros((C, B, R, B), np.float32)  # [c, i, r, o]
    vT = np.ascontiguousarray(values.transpose(0, 1, 3, 2))  # [r, k, i, o]
    for r in range(R):
        np.add.at(Wd[:, :, r, :], (col_indices[r],), vT[r])
    return Wd.reshape(D_IN, D_OUT)


def _run(x, values, bias, col_indices, trace=False):
    x = np.asarray(x, np.float32)
    values = np.asarray(values, np.float32)
    bias = np.asarray(bias, np.float32)
    col_indices = np.asarray(col_indices, np.int32)

    W = _scatter_dense(values, col_indices)  # [D_IN, D_OUT] fp32
    has_bias = bool(np.any(bias))
    FEAT = D_IN // 128 + (1 if has_bias else 0)

    # augment contraction with a bias row if needed
    xT = np.ascontiguousarray(x.T)  # [D_IN, N_TOK]
    if has_bias:
        xT = np.concatenate([xT, np.zeros((128, N_TOK), np.float32)], 0)
        xT[D_IN, :] = 1.0
        W = np.concatenate([W, np.zeros((128, D_OUT), np.float32)], 0)
        W[D_IN, :] = bias

    np_dt = ml_dtypes.bfloat16 if MM_DTYPE == "bf16" else np.float32
    # pre-tile W partition-major: [128, OGB*FEAT*256], cols
    # (ogb*FEAT + f)*256.. hold tile (ogb, f) so multi-f loads are
    # contiguous per partition
    Wt = np.ascontiguousarray(
        W.reshape(FEAT, 128, 16, 256).transpose(1, 2, 0, 3)
    ).reshape(128, 16 * FEAT * 256).astype(np_dt)
    xTc = xT.astype(np_dt)

    nc = _build_program(FEAT, MM_DTYPE)

    in_maps = []
    for c in range(NCORES):
        shard = xTc[:, c * TOK:(c + 1) * TOK]              # [FEAT*128, TOK]
        # partition-major: [128, FEAT*TOK], row p = partition p's data
        pm = np.ascontiguousarray(
            shard.reshape(FEAT, 128, TOK).transpose(1, 0, 2)
        ).reshape(128, FEAT * TOK)
        in_maps.append({"xT": pm, "W": Wt})

    res = run_bass_kernel_spmd(nc, in_maps, list(range(NCORES)), trace=trace)

    y = np.empty((N_TOK, D_OUT), np.float32)
    for c in range(NCORES):
        # yT: [128, OG*TOK] bf16, partition-major
        yT = res.results[c]["yT"].astype(np.float32).reshape(128, OG, TOK)
        # y[n, og*128+p] = yT[p, og, n]
        y[c * TOK:(c + 1) * TOK, :] = yT.transpose(2, 1, 0).reshape(TOK, D_OUT)
    return y, res


def kernel(x: np.ndarray, values: np.ndarray, bias: np.ndarray,
           col_indices: np.ndarray) -> np.ndarray:
    return _run(x, values, bias, col_indices)[0]


def run_traced(x, values, bias, col_indices):
    return _run(x, values, bias, col_indices, trace=True)[1]
